# revision 15
# baseline (speedup 1.0000x reference)
"""Trainium2 Bass kernel for nn_AConvCircular2D (4x128x48x48, 8 heads, dk=dv=128).

Sharding: core c handles batch b = c//2 and head-group g = c%2 (heads 4g..4g+3).
Per core:
  - 3x3 circular convs (init 64ch + q 64ch + k 64ch + v 64ch) as 9-tap matmul
    accumulation over a circularly padded x in SBUF (all bf16, fp32 accum).
  - attention for 4 heads: logitsT[m,n] tiles in PSUM -> ACT exp(0.25*x) -> E
    (bf16) -> attn@v with E as stationary operand; softmax sums come from a
    ones column appended to v^T; division folded into a post-matmul scale.
  - attn output stored (n, d)-row-major per head == the reference's raw
    reshape channel layout.
  - pair AllGather {2b, 2b+1} of the 64 attn channels -> full 128 channels.
  - 1x1 out-conv (64 out channels per core).
Output per core: (128, 2304) f32 = [64 init-conv channels | 64 attn-conv channels].
"""

import sys

sys.path.insert(0, "/opt/trn_rl_repo")

import numpy as np

import concourse.bass as bass  # noqa: F401
import concourse.mybir as mybir
from concourse import bacc, tile
from concourse.bass_utils import run_bass_kernel_spmd

F32 = mybir.dt.float32
BF16 = mybir.dt.bfloat16
AF = mybir.ActivationFunctionType

H = 48
W = 48
N = H * W            # 2304
NT = N // 128        # 18 n-tiles / m-tiles of 128
DH = 16              # per-head dim
HL = 4               # heads per core
SCALE = DH ** -0.5   # 0.25


def build(DEBUG=False):
    nc = bacc.Bacc("TRN2", target_bir_lowering=False, debug=False)

    x_ext = nc.declare_dram_parameter("x", [128, H, W], F32, isOutput=False)
    wc_ext = nc.declare_dram_parameter("wc", [9, 128, 256], F32, isOutput=False)
    wo_ext = nc.declare_dram_parameter("wo", [128, 64], F32, isOutput=False)
    out_ext = nc.declare_dram_parameter("out", [128, N], F32, isOutput=True)
    if DEBUG:
        dbg_qm = nc.declare_dram_parameter("dbg_qm", [128, N], F32, isOutput=True)
        dbg_vt = nc.declare_dram_parameter("dbg_vt", [128, HL * NT * 17], F32, isOutput=True)
        dbg_E = nc.declare_dram_parameter("dbg_E", [128, 1152], F32, isOutput=True)
        dbg_acc = nc.declare_dram_parameter("dbg_acc", [128, NT * 17], F32, isOutput=True)
        dbg_at = nc.declare_dram_parameter("dbg_at", [128, NT * DH], F32, isOutput=True)
        dbg_g = nc.declare_dram_parameter("dbg_g", [128, N], F32, isOutput=True)

    with tile.TileContext(nc) as tc:
        with (
            tc.tile_pool(name="const", bufs=1) as cp,
            tc.tile_pool(name="dram", bufs=1, space="DRAM") as dram,
        ):
            xp_bf = cp.tile([128, 50 * 50], BF16)    # circular-padded x
            wc_bf = cp.tile([128, 9 * 256], BF16)    # conv weights, tap-major
            wo_bf = cp.tile([128, 64], BF16)         # out-conv weights (in,out)
            qm_sb = cp.tile([128, N], BF16)          # [k_h | q_h] per 32-block
            kz_sb = cp.tile([128, N], BF16)          # [0 | k_h] per 32-block
            v_sb = cp.tile([128, N], BF16)           # rows 64..127 hold v (conv order)
            vv_sb = cp.tile([128, N], BF16)          # v_h at base 32h
            vt_sb = cp.tile([128, HL * NT * 17], BF16)  # v^T + ones col per m-tile
            g_sb = cp.tile([128, N], BF16)           # gathered attn channels
            id_bf = cp.tile([128, 16], BF16)  # identity replicated per 16-row block

            bounce = dram.tile([HL, NT, 128, DH], BF16)
            gath2 = dram.tile([2, 2, 2 * NT * 128 * DH], BF16)  # [pair, rank, 2-head block]

            # ---------------- load + prep ----------------
            with (
                tc.tile_pool(name="stage", bufs=2) as stage,
                tc.tile_pool(name="ps_a", bufs=3, space="PSUM") as ps_a,
            ):
                xp_f32 = stage.tile([128, 50 * 50], F32, tag="stage")
                xp2_f32 = xp_f32[:].rearrange("p (y x) -> p y x", y=50)
                # center + wrapped edges/corners (xp[y', x'] = x[(y'-1)%48, (x'-1)%48])
                nc.sync.dma_start(xp2_f32[:, 1:49, 1:49], x_ext[:, :, :])
                nc.sync.dma_start(xp2_f32[:, 1:49, 0:1], x_ext[:, :, 47:48])
                nc.sync.dma_start(xp2_f32[:, 1:49, 49:50], x_ext[:, :, 0:1])
                nc.sync.dma_start(xp2_f32[:, 0:1, 1:49], x_ext[:, 47:48, :])
                nc.sync.dma_start(xp2_f32[:, 49:50, 1:49], x_ext[:, 0:1, :])
                nc.sync.dma_start(xp2_f32[:, 0:1, 0:1], x_ext[:, 47:48, 47:48])
                nc.sync.dma_start(xp2_f32[:, 0:1, 49:50], x_ext[:, 47:48, 0:1])
                nc.sync.dma_start(xp2_f32[:, 49:50, 0:1], x_ext[:, 0:1, 47:48])
                nc.sync.dma_start(xp2_f32[:, 49:50, 49:50], x_ext[:, 0:1, 0:1])
                nc.vector.tensor_copy(xp_bf[:], xp_f32[:])

                wc_f32 = stage.tile([128, 9 * 256], F32, tag="stage")
                nc.sync.dma_start(
                    wc_f32[:].rearrange("p (t c) -> p t c", t=9), wc_ext[:].rearrange("t p c -> p t c")
                )
                nc.vector.tensor_copy(wc_bf[:], wc_f32[:])

                wo_f32 = stage.tile([128, 64], F32, tag="wo")
                nc.sync.dma_start(wo_f32[:], wo_ext[:])
                nc.vector.tensor_copy(wo_bf[:], wo_f32[:])

                nc.gpsimd.memset(id_bf[:], 0.0)
                nc.gpsimd.affine_select(
                    out=id_bf[0:16, :],
                    in_=id_bf[0:16, :],
                    compare_op=mybir.AluOpType.not_equal,
                    fill=1.0,
                    base=0,
                    pattern=[[-1, 16]],
                    channel_multiplier=1,
                )
                for blk in (2, 4, 6):
                    nc.sync.dma_start(id_bf[16 * blk : 16 * blk + 16, :], id_bf[0:16, :])
                nc.vector.memset(vt_sb[:], 1.0)

                # ---------------- main conv (q | init, k | v) ----------------
                xp2_bf = xp_bf[:].rearrange("p (y x) -> p y x", y=50)
                for mt in range(2):
                    for j in range(6):  # n-chunks of 384 (8 rows)
                        p = ps_a.tile([128, 384], F32, tag="conv")
                        for t in range(9):
                            dy, dx = t // 3, t % 3
                            rhs = xp2_bf[:, 8 * j + dy : 8 * j + dy + 8, dx : dx + 48]
                            nc.tensor.matmul(
                                p[:],
                                wc_bf[:, t * 256 + mt * 128 : t * 256 + (mt + 1) * 128],
                                rhs,
                                start=(t == 0),
                                stop=(t == 8),
                            )
                        sl = slice(384 * j, 384 * (j + 1))
                        if mt == 0:
                            nc.vector.tensor_copy(qm_sb[:, sl], p[:, :])
                        else:
                            ist = stage.tile([64, 384], F32, tag="ist")
                            nc.scalar.copy(ist[:, :], p[0:64, :])
                            nc.sync.dma_start(out_ext[0:64, sl], ist[:, :])
                            nc.vector.tensor_copy(v_sb[64:128, sl], p[64:128, :])

                # shift k into [0|k] stationary layout, v to 32h-aligned blocks
                nc.vector.memset(kz_sb[:], 0.0)
                for h in range(HL):
                    nc.sync.dma_start(
                        kz_sb[32 * h + 16 : 32 * h + 32, :], qm_sb[32 * h : 32 * h + 16, :]
                    )
                    nc.sync.dma_start(
                        vv_sb[32 * h : 32 * h + 16, :],
                        v_sb[64 + DH * h : 64 + DH * (h + 1), :],
                    )

                # ---------------- v^T via PE transpose ----------------
                for h in range(HL):
                    tp = ps_a.tile([128, NT * DH], BF16, tag="tp")
                    for t in range(NT):
                        nc.tensor.matmul(
                            tp[:, DH * t : DH * (t + 1)],
                            vv_sb[32 * h : 32 * h + DH, 128 * t : 128 * (t + 1)],
                            id_bf[32 * h : 32 * h + DH, :],
                            is_transpose=True,
                            start=True,
                            stop=True,
                            tile_position=(32 * h, 0),
                        )
                    dst = vt_sb[:].rearrange("p (h t c) -> p h t c", h=HL, t=NT)
                    nc.vector.tensor_copy(
                        dst[:, h, :, 0:DH],
                        tp[:].rearrange("p (t d) -> p t d", t=NT),
                    )

            # ---------------- attention (head pairs, concurrent PE row strips) ----------------
            with (
                tc.tile_pool(name="epool", bufs=3) as epool,
                tc.tile_pool(name="apool", bufs=2) as apool,
                tc.tile_pool(name="ps_log", bufs=2, space="PSUM") as ps_log,
                tc.tile_pool(name="ps_acc", bufs=2, space="PSUM") as ps_acc,
            ):
                for hp in range(HL // 2):
                    accs = []
                    for i in range(2):
                        acc_t = apool.tile([128, NT * 17], F32, tag=f"acc{i}", name=f"acc{i}")
                        accs.append(acc_t)
                    for mt in range(NT):
                        scrs = []
                        for i in range(2):
                            scr_t = ps_acc.tile([128, NT * 17], F32, tag="scr", name=f"scr{i}")
                            scrs.append(scr_t)
                        for half in range(2):
                            Ls = []
                            for i in range(2):
                                L_t = ps_log.tile([128, 1152], F32, tag="L", name=f"L{i}")
                                Ls.append(L_t)
                            # logits for both heads back-to-back: concurrent row strips
                            for (o, w_) in ((0, 512), (512, 512), (1024, 128)):
                                for i in range(2):
                                    h = 2 * hp + i
                                    nc.tensor.matmul(
                                        Ls[i][:, o : o + w_],
                                        kz_sb[32 * h : 32 * h + 32, 128 * mt : 128 * (mt + 1)],
                                        qm_sb[32 * h : 32 * h + 32,
                                              1152 * half + o : 1152 * half + o + w_],
                                        start=True,
                                        stop=True,
                                        tile_position=(32 * h, 0),
                                    )
                            for i in range(2):
                                h = 2 * hp + i
                                e_bf = epool.tile([128, 1152], BF16, tag="E")
                                nc.scalar.activation(e_bf[:], Ls[i][:], AF.Exp, scale=SCALE)
                                if DEBUG and h == 0 and mt == 0 and half == 0:
                                    dstg = epool.tile([128, 1152], F32, tag="dstg")
                                    nc.vector.tensor_copy(dstg[:], e_bf[:])
                                    nc.sync.dma_start(dbg_E[:], dstg[:])
                                for jj in range(9):
                                    gj = 9 * half + jj
                                    nc.tensor.matmul(
                                        scrs[i][:, 17 * gj : 17 * gj + 17],
                                        e_bf[:, 128 * jj : 128 * (jj + 1)],
                                        vt_sb[:, (h * NT + mt) * 17 : (h * NT + mt) * 17 + 17],
                                        start=True,
                                        stop=True,
                                    )
                        for i in range(2):
                            if mt == 0:
                                nc.vector.tensor_copy(accs[i][:], scrs[i][:])
                            else:
                                nc.vector.tensor_add(accs[i][:], scrs[i][:], accs[i][:])
                    for i in range(2):
                        h = 2 * hp + i
                        acc = accs[i]
                        if DEBUG and h == 0:
                            nc.sync.dma_start(dbg_acc[:], acc[:])
                        # softmax divide + store (n, d)-major
                        rec = apool.tile([128, NT], F32, tag="rec")
                        acc3 = acc[:].rearrange("p (t c) -> p t c", c=17)
                        nc.vector.reciprocal(rec[:], acc3[:, :, 16])
                        attn_bf = apool.tile([128, NT * DH], BF16, tag="attn")
                        at3 = attn_bf[:].rearrange("p (t d) -> p t d", t=NT)
                        for t in range(NT):
                            nc.vector.tensor_scalar_mul(
                                at3[:, t, :], acc3[:, t, 0:DH], rec[:, t : t + 1]
                            )
                        nc.sync.dma_start(
                            bounce[h].rearrange("t p d -> p t d"),
                            at3[:, :, :],
                        )
                        if DEBUG and h == 0:
                            dstg3 = epool.tile([128, NT * DH], F32, tag="dstg3")
                            nc.vector.tensor_copy(dstg3[:], attn_bf[:])
                            nc.sync.dma_start(dbg_at[:], dstg3[:])
                    # per-head-pair gather, overlapped with next pair's compute
                    nc.gpsimd.collective_compute(
                        "AllGather",
                        mybir.AluOpType.bypass,
                        replica_groups=[[0, 1], [2, 3], [4, 5], [6, 7]],
                        ins=[bounce[2 * hp : 2 * hp + 2].opt()],
                        outs=[gath2[hp].opt()],
                    )

            # ---------------- 1x1 out conv ----------------
            with (
                tc.tile_pool(name="ps_o", bufs=2, space="PSUM") as ps_o,
                tc.tile_pool(name="ost", bufs=2) as ost,
            ):
                # g_sb rows = attn channels 0..127; gath2[hp][r] holds channels
                # [64r + 32hp, 64r + 32hp + 32) as (32, 2304) row-major
                for hp in range(2):
                    for r in range(2):
                        nc.sync.dma_start(
                            g_sb[64 * r + 32 * hp : 64 * r + 32 * hp + 32, :],
                            gath2[hp, r].rearrange("(c n) -> c n", c=32),
                        )
                if DEBUG:
                    gstg = ost.tile([128, N], F32, tag="gstg")
                    nc.vector.tensor_copy(gstg[:], g_sb[:])
                    nc.sync.dma_start(dbg_g[:], gstg[:])
                    qstg = ost.tile([128, N], F32, tag="qstg")
                    nc.vector.tensor_copy(qstg[:], qm_sb[:])
                    nc.sync.dma_start(dbg_qm[:], qstg[:])
                    vtstg = ost.tile([128, HL * NT * 17], F32, tag="vtstg")
                    nc.vector.tensor_copy(vtstg[:], vt_sb[:])
                    nc.sync.dma_start(dbg_vt[:], vtstg[:])
                for j in range(6):
                    po = ps_o.tile([64, 384], F32, tag="oc")
                    nc.tensor.matmul(
                        po[:], wo_bf[:], g_sb[:, 384 * j : 384 * (j + 1)],
                        start=True, stop=True,
                    )
                    ot = ost.tile([64, 384], F32, tag="ot")
                    nc.scalar.copy(ot[:], po[:])
                    nc.sync.dma_start(out_ext[64:128, 384 * j : 384 * (j + 1)], ot[:])

    nc.compile()
    return nc


_NC_CACHE = None


def _get_nc(DEBUG=False):
    global _NC_CACHE
    if _NC_CACHE is None:
        _NC_CACHE = build(DEBUG)
    return _NC_CACHE


def _shard_inputs(x, w_init, w_qkv, w_out):
    in_maps = []
    taps = lambda w: w.reshape(w.shape[0], 128, 9)  # (O, I, 3, 3) -> (O, I, 9)
    wi, wq, wo = taps(w_init), taps(w_qkv), w_out[:, :, 0, 0]
    for c in range(8):
        b, g = c // 2, c % 2
        s = slice(64 * g, 64 * (g + 1))
        # mt0 cols: [k_h(16) | q_h(16)] x 4 heads; mt1 cols: [init 64 | v 64]
        wc = np.empty((9, 128, 256), np.float32)
        for h in range(4):
            wc[:, :, 32 * h : 32 * h + 16] = wq[128 + 64 * g + 16 * h : 128 + 64 * g + 16 * (h + 1)].transpose(2, 1, 0)
            wc[:, :, 32 * h + 16 : 32 * h + 32] = wq[64 * g + 16 * h : 64 * g + 16 * (h + 1)].transpose(2, 1, 0)
        wc[:, :, 128:192] = wi[s].transpose(2, 1, 0)
        wc[:, :, 192:256] = wq[256 + 64 * g : 256 + 64 * (g + 1)].transpose(2, 1, 0)
        in_maps.append(
            {
                "x": np.ascontiguousarray(x[b], np.float32),
                "wc": np.ascontiguousarray(wc),
                "wo": np.ascontiguousarray(wo[s].T, dtype=np.float32),
            }
        )
    return in_maps


def kernel(x, w_init, w_qkv, w_out, _trace=False, _debug=False):
    nc = _get_nc(_debug)
    in_maps = _shard_inputs(
        np.asarray(x, np.float32),
        np.asarray(w_init, np.float32),
        np.asarray(w_qkv, np.float32),
        np.asarray(w_out, np.float32),
    )
    res = run_bass_kernel_spmd(nc, in_maps, core_ids=list(range(8)), trace=_trace)
    if _debug:
        return res
    full = np.empty((4, 256, 48, 48), np.float32)
    for c in range(8):
        b, g = c // 2, c % 2
        o = res.results[c]["out"].reshape(128, 48, 48)
        full[b, 64 * g : 64 * (g + 1)] = o[0:64]
        full[b, 128 + 64 * g : 128 + 64 * (g + 1)] = o[64:128]
    if _trace:
        return full, res
    return full
